# revision 3
# baseline (speedup 1.0000x reference)
"""Trainium2 Bass kernel for nn_EPAN_8735963480604 (sparse_attention).

Reference computation (per batch b, heads h=8, d=64, N=4096, C=512, P=64):
  qkv = x @ w_qkv.T                       -> q,k,v  [H,d,N] views
  k_proj = k @ w_e.T + b_e                -> [H,d,P]   (shared E/F linear)
  v_proj = v @ w_e.T + b_e                -> [H,d,P]
  q = l2normalize(q, axis=N)
  attn = softmax((q^T k_proj) * temperature, axis=P)
  out = attn @ v_proj^T                   -> [H,N,d] -> permute/reshape

Sharding: batch B=8, one batch element per NeuronCore (data parallel),
weights replicated. Each core computes its full batch element.

Key implementation choices:
 - fp32r (TF32-like, ~1.5e-4 rel rms err) matmuls at 1 PE cycle/row for all
   free-dim>=256 stages; fp32 for the small PV stage.
 - x and w_qkv transposed on-chip via PE transpose-mode matmuls (contraction
   dim must sit on SBUF partitions).
 - q-normalization + temperature folded into k_proj columns (scores are
   linear in k_proj), so only a [d,P] tile gets scaled, not q itself.
 - softmax denominator via an appended ones-column on v_proj: one fused
   matmul produces [PV | Z]; normalize with per-partition reciprocal.
 - b_e bias applied during PSUM->SBUF eviction on the scalar engine.
"""

import sys

sys.path.insert(0, "/opt/trn_rl_repo")

import numpy as np

N_CORES = 8
B, N, C = 8, 4096, 512
H, D = 8, 64          # heads, head dim
PP = 64               # projection dim P
NT = N // 128         # 32 n-tiles of 128
CB = C // 128         # 4 c-tiles of 128
G = H // 2            # 4 head pairs
NJ = N // 512         # 8 n-chunks of 512

_cache = {}


def _build():
    import concourse.bacc as bacc
    import concourse.mybir as mybir
    import concourse.tile as tile

    f32 = mybir.dt.float32
    f32r = mybir.dt.float32r
    AF = mybir.ActivationFunctionType

    nc = bacc.Bacc("TRN2", target_bir_lowering=False, debug=False,
                   num_devices=N_CORES)

    xb_d = nc.dram_tensor("xb", [N, C], f32, kind="ExternalInput").ap()
    wqkv_d = nc.dram_tensor("wqkv", [3 * C, C], f32, kind="ExternalInput").ap()
    we_d = nc.dram_tensor("we", [PP, N], f32, kind="ExternalInput").ap()
    be_d = nc.dram_tensor("be2", [PP, 1], f32, kind="ExternalInput").ap()
    temp_d = nc.dram_tensor("temp4", [128, G], f32, kind="ExternalInput").ap()
    id_d = nc.dram_tensor("ident", [128, 128], f32, kind="ExternalInput").ap()
    ob_d = nc.dram_tensor("ob", [N, C], f32, kind="ExternalOutput").ap()

    from contextlib import ExitStack

    with tile.TileContext(nc) as tc, ExitStack() as ctx:
            const = ctx.enter_context(tc.tile_pool(name="const", bufs=1))
            wqs = ctx.enter_context(tc.tile_pool(name="wqs", bufs=2))
            wqTp = ctx.enter_context(tc.tile_pool(name="wqT", bufs=1))
            weTp = ctx.enter_context(tc.tile_pool(name="weT", bufs=1))
            xs = ctx.enter_context(tc.tile_pool(name="xs", bufs=3))
            xTp = ctx.enter_context(tc.tile_pool(name="xT", bufs=1))
            kvp_pool = ctx.enter_context(tc.tile_pool(name="kv", bufs=3))
            qTp = ctx.enter_context(tc.tile_pool(name="qT", bufs=2))
            sqp = ctx.enter_context(tc.tile_pool(name="sq", bufs=1))
            Ebp = ctx.enter_context(tc.tile_pool(name="Eb", bufs=3))
            osbp = ctx.enter_context(tc.tile_pool(name="osb", bufs=3))
            vpop = ctx.enter_context(tc.tile_pool(name="vpo", bufs=1))
            kpTsp = ctx.enter_context(tc.tile_pool(name="kpTs", bufs=1))
            kpgp = ctx.enter_context(tc.tile_pool(name="kpg", bufs=2))
            small = ctx.enter_context(tc.tile_pool(name="small", bufs=8))
            ident = const.tile([128, 128], f32r)
            be_sb = const.tile([PP, 1], f32)
            temp_sb = const.tile([128, G], f32)
            nc.sync.dma_start(ident[:], id_d[:].bitcast(f32r))
            nc.sync.dma_start(be_sb[:], be_d[:])
            nc.sync.dma_start(temp_sb[:], temp_d[:])

            we_s = const.tile([PP, N], f32r)
            nc.sync.dma_start(we_s[:], we_d[:].bitcast(f32r))

            wqT = [wqTp.tile([128, 3 * C], f32r, tag=f"wqT{cb}", name=f"wqT{cb}") for cb in range(CB)]
            weT = weTp.tile([128, NT, PP], f32r)
            xT = [xTp.tile([128, N], f32r, tag=f"xT{cb}", name=f"xT{cb}") for cb in range(CB)]

            with ExitStack() as ph12:
                ps_t = ph12.enter_context(tc.tile_pool(name="ps_t", bufs=2, space="PSUM"))
                ps_a2 = ph12.enter_context(tc.tile_pool(name="ps_a2", bufs=2, space="PSUM"))
                ps_b = ph12.enter_context(tc.tile_pool(name="ps_b", bufs=1, space="PSUM"))
                # ---- Phase 0: transpose w_e -> weT [n,p] tiles --------------
                for i in range(NT):
                    tp = ps_t.tile([128, PP], f32r, tag="tp")
                    nc.tensor.transpose(
                        tp[:], we_s[:, i * 128:(i + 1) * 128], ident[0:PP, 0:PP]
                    )
                    nc.vector.tensor_copy(weT[:, i, :], tp[:])

                # ---- Phase 0b: transpose w_qkv -> wqT [c, 3C] ---------------
                for t in range(3 * C // 128):
                    wq_t = wqs.tile([128, C], f32r)
                    nc.sync.dma_start(
                        wq_t[:], wqkv_d[t * 128:(t + 1) * 128, :].bitcast(f32r)
                    )
                    for cb in range(CB):
                        tp = ps_t.tile([128, 128], f32r, tag="tp")
                        nc.tensor.transpose(
                            tp[:], wq_t[:, cb * 128:(cb + 1) * 128], ident[:]
                        )
                        nc.vector.tensor_copy(
                            wqT[cb][:, t * 128:(t + 1) * 128], tp[:]
                        )

                # ---- Phase 1+2: stream x: transpose, project k|v, reduce ----
                kpT_ps = ps_b.tile([PP, C], f32, tag="kpT")
                vpT_ps = ps_b.tile([PP, C], f32, tag="vpT")
                for i in range(NT):
                    x_t = xs.tile([128, C], f32r)
                    nc.sync.dma_start(
                        x_t[:], xb_d[i * 128:(i + 1) * 128, :].bitcast(f32r)
                    )
                    for cb in range(CB):
                        tp = ps_t.tile([128, 128], f32r, tag="tp")
                        nc.tensor.transpose(
                            tp[:], x_t[:, cb * 128:(cb + 1) * 128], ident[:]
                        )
                        nc.scalar.copy(xT[cb][:, i * 128:(i + 1) * 128], tp[:])
                    # kv chunk: [128n, 1024] = x_chunk @ w_qkv.T (k|v cols)
                    kv_ps = ps_a2.tile([128, 2 * C], f32)
                    for cb in range(CB):
                        for f in range(2):
                            nc.tensor.matmul(
                                kv_ps[:, f * C:(f + 1) * C],
                                xT[cb][:, i * 128:(i + 1) * 128],
                                wqT[cb][:, C + f * C:C + (f + 1) * C],
                                start=(cb == 0),
                                stop=(cb == CB - 1),
                            )
                    kv_t = kvp_pool.tile([128, 2 * C], f32r)
                    nc.vector.tensor_copy(kv_t[:], kv_ps[:])
                    # accumulate k_proj^T, v_proj^T  [p, (h d)]
                    nc.tensor.matmul(
                        kpT_ps[:], weT[:, i, :], kv_t[:, 0:C],
                        start=(i == 0), stop=(i == NT - 1),
                    )
                    nc.tensor.matmul(
                        vpT_ps[:], weT[:, i, :], kv_t[:, C:2 * C],
                        start=(i == 0), stop=(i == NT - 1),
                    )

                # ---- Phase 2.5: evict kpT/vpT with bias -------------------
                kpTs = kpTsp.tile([PP, C], f32r)
                nc.scalar.activation(
                    kpTs[:], kpT_ps[:], AF.Identity, bias=be_sb[:, 0:1]
                )
                vpo = vpop.tile([PP, H * (D + 1)], f32)
                nc.vector.memset(vpo[:], 1.0)
                for h in range(H):
                    nc.scalar.activation(
                        vpo[:, h * (D + 1):h * (D + 1) + D],
                        vpT_ps[:, h * D:(h + 1) * D],
                        AF.Identity, bias=be_sb[:, 0:1],
                    )

            # ---- Phase 3: per head-pair: q^T, norms, scores, softmax, PV ----
            with ExitStack() as ph3:
                ps_a1 = ph3.enter_context(tc.tile_pool(name="ps_a1", bufs=2, space="PSUM"))
                ps_c = ph3.enter_context(tc.tile_pool(name="ps_c", bufs=1, space="PSUM"))
                ps_d = ph3.enter_context(tc.tile_pool(name="ps_d", bufs=2, space="PSUM"))
                ps_kp = ph3.enter_context(tc.tile_pool(name="ps_kp", bufs=1, space="PSUM"))
                for g in range(G):
                    # qT_g [128 (2 heads x d), N]
                    qTg = qTp.tile([128, N], f32r)
                    for j in range(NJ):
                        qp = ps_a1.tile([128, 512], f32, tag="qp")
                        for cb in range(CB):
                            nc.tensor.matmul(
                                qp[:],
                                wqT[cb][:, g * 128:(g + 1) * 128],
                                xT[cb][:, j * 512:(j + 1) * 512],
                                start=(cb == 0), stop=(cb == CB - 1),
                            )
                        nc.vector.tensor_copy(qTg[:, j * 512:(j + 1) * 512], qp[:])
                    # row norms of q (over N), fold temperature
                    sq_t = sqp.tile([128, N], f32)
                    n2 = small.tile([128, 1], f32, tag="n2")
                    nc.scalar.activation(
                        sq_t[:], qTg[:].bitcast(f32), AF.Square, accum_out=n2[:]
                    )
                    rcp = small.tile([128, 1], f32, tag="rcp")
                    nc.vector.reciprocal(rcp[:], n2[:])
                    invn = small.tile([128, 1], f32, tag="invn")
                    nc.scalar.sqrt(invn[:], rcp[:])
                    sv = small.tile([128, 1], f32, tag="sv")
                    nc.vector.tensor_mul(sv[:], invn[:], temp_sb[:, g:g + 1])
                    # kp pair tile [128 (2 heads x d), PP], scaled
                    kpp = ps_kp.tile([128, PP], f32r, tag="kpp")
                    nc.tensor.transpose(
                        kpp[:], kpTs[:, g * 128:(g + 1) * 128], ident[0:PP, 0:PP]
                    )
                    kpg = kpgp.tile([128, PP], f32r)
                    nc.vector.tensor_scalar_mul(kpg[:], kpp[:], sv[:, 0:1])

                    for j in range(NJ):
                        cp0 = ps_c.tile([PP, 512], f32, tag="cp0")
                        cp1 = ps_c.tile([PP, 512], f32, tag="cp1")
                        nc.tensor.matmul(
                            cp0[:], kpg[0:PP, :],
                            qTg[0:PP, j * 512:(j + 1) * 512],
                            start=True, stop=True,
                        )
                        nc.tensor.matmul(
                            cp1[:], kpg[PP:128, :],
                            qTg[PP:128, j * 512:(j + 1) * 512],
                            start=True, stop=True,
                        )
                        E0 = Ebp.tile([PP, 512], f32, tag="E0")
                        E1 = Ebp.tile([PP, 512], f32, tag="E1")
                        nc.scalar.activation(E0[:], cp0[:], AF.Exp)
                        nc.scalar.activation(E1[:], cp1[:], AF.Exp)
                        for js in range(4):
                            dp = ps_d.tile([128, 2 * (D + 1)], f32, tag="dp")
                            nc.tensor.matmul(
                                dp[:, 0:D + 1],
                                E0[:, js * 128:(js + 1) * 128],
                                vpo[:, (2 * g) * (D + 1):(2 * g + 1) * (D + 1)],
                                start=True, stop=True,
                            )
                            nc.tensor.matmul(
                                dp[:, D + 1:2 * (D + 1)],
                                E1[:, js * 128:(js + 1) * 128],
                                vpo[:, (2 * g + 1) * (D + 1):(2 * g + 2) * (D + 1)],
                                start=True, stop=True,
                            )
                            z0 = small.tile([128, 1], f32, tag="z0")
                            z1 = small.tile([128, 1], f32, tag="z1")
                            nc.vector.reciprocal(z0[:], dp[:, D:D + 1])
                            nc.vector.reciprocal(z1[:], dp[:, 2 * D + 1:2 * D + 2])
                            ot = osbp.tile([128, 128], f32)
                            nc.vector.tensor_scalar_mul(
                                ot[:, 0:D], dp[:, 0:D], z0[:, 0:1]
                            )
                            nc.vector.tensor_scalar_mul(
                                ot[:, D:2 * D], dp[:, D + 1:2 * D + 1], z1[:, 0:1]
                            )
                            n0 = (j * 4 + js) * 128
                            nc.sync.dma_start(
                                ob_d[n0:n0 + 128, g * 128:(g + 1) * 128], ot[:]
                            )

    nc.compile()
    return nc


def _get_nc():
    if "nc" not in _cache:
        _cache["nc"] = _build()
    return _cache["nc"]


def kernel(x, w_qkv, w_e, b_e, temperature):
    import os

    from concourse import bass_utils

    nc = _get_nc()
    trace = bool(os.environ.get("KERNEL_TRACE"))

    x = np.asarray(x, dtype=np.float32)
    w_qkv = np.ascontiguousarray(np.asarray(w_qkv, dtype=np.float32))
    w_e = np.ascontiguousarray(np.asarray(w_e, dtype=np.float32))
    b_e2 = np.ascontiguousarray(np.asarray(b_e, dtype=np.float32).reshape(PP, 1))
    # temp4[p, g] = temperature[2g + p//64]
    trep = np.repeat(np.asarray(temperature, dtype=np.float32).reshape(H), D)
    temp4 = np.ascontiguousarray(trep.reshape(G, 128).T)
    ident = np.eye(128, dtype=np.float32)

    in_maps = []
    for c in range(N_CORES):
        in_maps.append({
            "xb": np.ascontiguousarray(x[c]),
            "wqkv": w_qkv,
            "we": w_e,
            "be2": b_e2,
            "temp4": temp4,
            "ident": ident,
        })

    res = bass_utils.run_bass_kernel_spmd(
        nc, in_maps, core_ids=list(range(N_CORES)), trace=trace
    )
    _cache["last_res"] = res

    # per-core ob is [N, H*D] with out[b,h,n,d] at ob[n, h*64+d].
    # reference returns out.transpose(0,3,1,2).reshape(B,N,C).
    out = np.empty((B, N, C), dtype=np.float32)
    for c in range(N_CORES):
        ob = res.results[c]["ob"]
        out[c] = np.ascontiguousarray(
            ob.reshape(N, H, D).transpose(2, 1, 0)
        ).reshape(N, C)
    return out

